# revision 3
# baseline (speedup 1.0000x reference)
"""Bass/Tile kernel for a single causal attention head on 8 trn2 NeuronCores.

Problem: input [8, 2048, 768], Wq/Wk/Wv [768, 64] ->
  O = softmax(causal(Q K^T)/sqrt(64)) V, per batch.  [8, 2048, 64]

Sharding: data-parallel over batch; core b handles batch b. Weights replicated.

v2: all-bf16 matmul pipeline (tolerance 2e-2; measured ~4e-3).
  - bf16 matmuls run 1 cyc/row incl. <256-wide outputs (fp32r degrades to 4),
    bf16 PE transposes run 1 cyc/row (fp32 takes 2).
  - default front end (ATTN_X=f32): x stays f32 in DRAM; a gpsimd
    (software-DGE) DMA casts f32->bf16 in flight while loading to SBUF;
    PE transposes the bf16 tiles (psum bf16) and DVE copies to xT at 2x.
  - ATTN_X=bf16: host-casts x, xbar DMA-transpose straight from DRAM
    (no PE transposes at all).

Per-core dataflow (same skeleton as v1):
  1. xT [768, 2048] bf16 in SBUF (via one of the front ends above).
  2. Projections with concatenated stationary weights [Wq|Wk], [Wv|Wq]:
     each [128, 512] psum f32 -> cast-copy to QK_sb / VQ_sb bf16.
     V^T tiles PE-transposed back to V_aug [128, 65] bf16 (ones col makes
     the O^T matmul emit softmax denominators for free).
  3. Attention per 512-wide query range: S^T pair matmuls (bf16 in, f32
     psum), one exp per pair on ACT (1/8 scale fused, bf16 out), causal
     zeroing of diagonal blocks via gpsimd affine_select, O^T accumulation
     over k in psum.
  4. O^T+rowsum PE-transposed back to [128, 65] f32; out = O * (1/rowsum).

QK_sb/VQ_sb/Vb are double-buffered across reps so rep r+1's projections
don't stall on rep r's attention tail.
"""

import os
import numpy as np

import concourse.tile as tile
from concourse import bacc, mybir
from concourse.bass_utils import run_bass_kernel_spmd
from concourse.masks import make_identity

P = 128
N = 2048
D = 768
H = 64
NT = N // P   # 16 n-tiles
DT = D // P   # 6 d-tiles
W = 512       # q-range width
QR = N // W   # 4 q-ranges
F32 = mybir.dt.float32
BF = mybir.dt.bfloat16

# bf16 x-path: host-cast x to bf16, xbar DMA-transpose straight from DRAM.
BF16_X = os.environ.get("ATTN_X", "f32") == "bf16"

# Three of the four diagonal tiles (widths 512, 384, 128) pack into one
# [128, 1024] psum (bank0: 512 | bank1: 384+128); the 256-wide one (jj=2)
# gets its own narrow tile.  No matmul output crosses a 512-col f32 bank.
DIAG_PACK = {0: (0, 512), 1: (512, 384), 3: (896, 128)}  # jj -> (off, width)
DIAG_TOT = 1024


def build_kernel(reps=1):
    nc = bacc.Bacc(name="attn_head")
    x_d = nc.dram_tensor("x", [N, D], BF if BF16_X else F32,
                         kind="ExternalInput")
    wq_d = nc.dram_tensor("Wq", [D, H], F32, kind="ExternalInput")
    wk_d = nc.dram_tensor("Wk", [D, H], F32, kind="ExternalInput")
    wv_d = nc.dram_tensor("Wv", [D, H], F32, kind="ExternalInput")
    out_d = nc.dram_tensor("out", [N, H], F32, kind="ExternalOutput")

    Exp = mybir.ActivationFunctionType.Exp

    with tile.TileContext(nc) as tc:
        with (
            tc.tile_pool(name="persist", bufs=1) as persist,
            tc.tile_pool(name="xload", bufs=2) as xload,
            tc.tile_pool(name="work", bufs=3) as work,
            tc.tile_pool(name="psum", bufs=1, space="PSUM") as psum,
        ):
            # warm the ACT exp table while DMAs run
            dummy = persist.tile([P, 1], F32)
            nc.vector.memset(dummy[:], 0.0)
            nc.scalar.activation(dummy[:], dummy[:], Exp)

            ident = persist.tile([P, P], F32)
            make_identity(nc, ident[:])
            ident_bf = persist.tile([P, P], BF)
            nc.vector.tensor_copy(out=ident_bf[:], in_=ident[:])

            ones_col = persist.tile([P, 1], F32)
            nc.vector.memset(ones_col[:], 1.0)

            xT = persist.tile([P, DT, N], BF)        # x^T: [d%128, d//128, n]
            QK_sb = persist.tile([P, 2, N], BF)      # rows 0-63 Q^T, 64-127 K^T
            VQ_sb = persist.tile([P, 2, N], BF)      # rows 0-63 V^T, 64-127 Q^T
            Vb = persist.tile([P, 2, NT, H + 1], BF)  # V tiles + ones col
            for s in range(2):
                nc.vector.tensor_copy(
                    out=Vb[:, s, :, H],
                    in_=ones_col[:, 0].to_broadcast((P, NT)),
                )

            # weights as [128, 6, 64]: partition = d%128, tile = d//128.
            # Concatenated pairs [Wq|Wk] and [Wv|Wq] make 128-wide stationary
            # operands: one projection matmul computes two 64-row outputs.
            w_raw = {}
            w_dma = []
            for wname, dram in (("q", wq_d), ("k", wk_d), ("v", wv_d)):
                w_raw[wname] = persist.tile([P, DT, H], F32,
                                            name=f"wraw_{wname}")
                w_dma.append((w_raw[wname], dram))
            w_qk = persist.tile([P, DT, 2 * H], BF)
            w_vq = persist.tile([P, DT, 2 * H], BF)

            def load_weights():
                for w_t, dram in w_dma:
                    nc.sync.dma_start(
                        out=w_t[:],
                        in_=dram[:, :].rearrange("(t p) h -> p t h", p=P),
                    )
                nc.vector.tensor_copy(out=w_qk[:, :, 0:H], in_=w_raw["q"][:])
                nc.vector.tensor_copy(out=w_qk[:, :, H:], in_=w_raw["k"][:])
                nc.vector.tensor_copy(out=w_vq[:, :, 0:H], in_=w_raw["v"][:])
                nc.vector.tensor_copy(out=w_vq[:, :, H:], in_=w_raw["q"][:])

            for rep in range(reps):
                sl = rep % 2
                # ---- x load + transpose to xT ------------------------------
                if BF16_X:
                    # x arrives bf16: xbar DMA-transpose straight from DRAM,
                    # chunked by query range so projections start early
                    for g in range(QR):
                        ns = slice(g * W, (g + 1) * W)
                        for d_i in range(DT):
                            nc.sync.dma_start(
                                out=xT[:, d_i, ns],
                                in_=x_d[ns, d_i * P:(d_i + 1) * P],
                                transpose=True,
                            )
                    if rep == 0:
                        load_weights()
                else:
                    # casting DMA (gpsimd software DGE): f32 DRAM -> bf16
                    # SBUF, one DMA per 4-tile group for pipelining
                    xbs = []
                    for g in range(QR):
                        xb = xload.tile([P, 4, D], BF, tag="xb",
                                        name=f"xb_{rep}_{g}", bufs=2)
                        xbs.append(xb)
                        nc.gpsimd.dma_start(
                            out=xb[:],
                            in_=x_d[g * W:(g + 1) * W, :].rearrange(
                                "(t p) d -> p t d", p=P),
                        )
                    if rep == 0:
                        load_weights()
                    # PE transpose bf16 (1 cyc/row), psum bf16, DVE 2x copy
                    for g in range(QR):
                        for d_i in range(DT):
                            pt = psum.tile([P, 4, P], BF, tag="mm", bufs=2)
                            for i in range(4):
                                nc.tensor.transpose(
                                    pt[:, i, :],
                                    xbs[g][:, i, d_i * P:(d_i + 1) * P],
                                    ident_bf[:],
                                )
                            nc.vector.tensor_copy(
                                out=xT[:, d_i, g * W:(g + 1) * W],
                                in_=pt[:])

                # ---- projections (packed via concatenated weights) ---------
                for r in range(QR):
                    ns = slice(r * W, (r + 1) * W)
                    pqk = psum.tile([P, W], F32, tag="proj", bufs=2)
                    pvq = psum.tile([P, W], F32, tag="proj", bufs=2)
                    for d_i in range(DT):
                        kw = dict(start=(d_i == 0), stop=(d_i == DT - 1))
                        rhs = xT[:, d_i, ns]
                        nc.tensor.matmul(pqk[:], w_qk[:, d_i], rhs, **kw)
                        nc.tensor.matmul(pvq[:], w_vq[:, d_i], rhs, **kw)
                    nc.scalar.copy(out=QK_sb[:, sl, ns], in_=pqk[:])
                    nc.vector.tensor_copy(out=VQ_sb[:, sl, ns], in_=pvq[:])
                    # V^T tiles -> V_aug [128, 65] per k-tile (batched copy)
                    pvt = psum.tile([P, 4, H], BF, tag="mm", bufs=2)
                    for i in range(4):
                        nc.tensor.transpose(
                            pvt[:, i, :],
                            VQ_sb[0:H, sl, r * W + i * P:r * W + (i + 1) * P],
                            ident_bf[:H, :H],
                        )
                    nc.vector.tensor_copy(out=Vb[:, sl, 4 * r:4 * r + 4, 0:H],
                                          in_=pvt[:])

                # ---- attention per q-range ---------------------------------
                KTd = QK_sb[H:P, sl, :]   # K^T on partitions 64-127
                QTd = VQ_sb[H:P, sl, :]   # Q^T duplicate on partitions 64-127
                for r in range(QR):
                    po = psum.tile([H + 1, W], F32, tag="po", bufs=2)
                    qs = slice(r * W, (r + 1) * W)
                    last_r = (r == QR - 1)

                    def do_pairs(first, last):
                        for jp in range(2 * r):
                            ps2 = psum.tile([P, 2, W], F32, tag="mm", bufs=2,
                                            name=f"ps2_{rep}_{r}_{jp}")
                            es2 = work.tile([P, 2, W], BF, tag="es2", bufs=6,
                                            name=f"es2_{rep}_{r}_{jp}")
                            for u in range(2):
                                j = 2 * jp + u
                                nc.tensor.matmul(
                                    ps2[:, u, :], KTd[:, j * P:(j + 1) * P],
                                    QTd[:, qs], start=True, stop=True,
                                )
                            nc.scalar.activation(es2[:], ps2[:], Exp,
                                                 scale=0.125)
                            for u in range(2):
                                j = 2 * jp + u
                                nc.tensor.matmul(
                                    po[:], Vb[:, sl, j, :], es2[:, u, :],
                                    start=(first and j == 0),
                                    stop=(last and jp == 2 * r - 1 and u == 1),
                                    skip_group_check=True,
                                )

                    if not last_r:
                        do_pairs(first=True, last=False)

                    # diagonal k-tiles: 3 packed in [128, 1024] + 1 [128, 256]
                    psd = psum.tile([P, DIAG_TOT], F32, tag="mm", bufs=2)
                    esd = work.tile([P, DIAG_TOT], BF, tag="esd", bufs=4)
                    ps1 = psum.tile([P, 256], F32, tag="proj", bufs=2)
                    es1 = work.tile([P, 256], BF, tag="es1", bufs=4)
                    for jj in range(4):
                        j = 4 * r + jj
                        if jj == 2:
                            sv = ps1[:, :]
                        else:
                            poff, wd = DIAG_PACK[jj]
                            sv = psd[:, poff:poff + wd]
                        nc.tensor.matmul(
                            sv,
                            KTd[:, j * P:(j + 1) * P],
                            QTd[:, r * W + jj * P:(r + 1) * W],
                            start=True, stop=True,
                        )
                    nc.scalar.activation(esd[:, 0:W], psd[:, 0:W], Exp,
                                         scale=0.125)
                    nc.scalar.activation(esd[:, W:], psd[:, W:], Exp,
                                         scale=0.125)
                    nc.scalar.activation(es1[:], ps1[:], Exp, scale=0.125)
                    # zero the invalid (q < k) half of each diagonal 128x128
                    # block post-exp, on the otherwise-idle GPSIMD engine
                    for jj in range(4):
                        ev = es1[:, 0:P] if jj == 2 else (
                            esd[:, DIAG_PACK[jj][0]:DIAG_PACK[jj][0] + P])
                        nc.gpsimd.affine_select(
                            out=ev, in_=ev,
                            compare_op=mybir.AluOpType.is_ge,
                            fill=0.0, base=0,
                            pattern=[[1, P]], channel_multiplier=-1,
                        )
                    for jj in range(4):
                        j = 4 * r + jj
                        if jj == 2:
                            rhs = es1[:, :]
                        else:
                            poff, wd = DIAG_PACK[jj]
                            rhs = esd[:, poff:poff + wd]
                        nc.tensor.matmul(
                            po[:, jj * P:],
                            Vb[:, sl, j, :],
                            rhs,
                            start=((r == 0 or last_r) and jj == 0),
                            stop=(not last_r and jj == 3),
                            skip_group_check=True,
                        )

                    if last_r:
                        do_pairs(first=False, last=True)

                    # ---- normalize + output (pipelined per n-tile) ---------
                    ot = work.tile([H + 1, W], F32, tag="ot", bufs=4)
                    nc.vector.tensor_copy(out=ot[:], in_=po[:])
                    pf = psum.tile([P, 4, H + 1], F32, tag="proj", bufs=2)
                    for i in range(4):
                        nt = r * 4 + i
                        nc.tensor.transpose(
                            pf[:, i, :], ot[:, i * P:(i + 1) * P],
                            ident[:H + 1, :H + 1],
                        )
                        rs = work.tile([P, 1], F32, tag="rs",
                                       name=f"rs_{rep}_{nt}")
                        nc.vector.reciprocal(rs[:], pf[:, i, H:H + 1])
                        ob = work.tile([P, H], F32, tag="ob",
                                       name=f"ob_{rep}_{nt}", bufs=4)
                        nc.vector.tensor_scalar_mul(
                            ob[:], pf[:, i, 0:H], rs[:]
                        )
                        nc.sync.dma_start(
                            out=out_d[nt * P:(nt + 1) * P, :],
                            in_=ob[:],
                        )

    nc.compile()
    return nc


_NC_CACHE = {}


def _get_nc(reps=1):
    if reps not in _NC_CACHE:
        _NC_CACHE[reps] = build_kernel(reps)
    return _NC_CACHE[reps]


def kernel(input, Wq, Wk, Wv, **_unused):
    if BF16_X:
        import ml_dtypes
        input = np.ascontiguousarray(
            np.asarray(input).astype(ml_dtypes.bfloat16))
    else:
        input = np.ascontiguousarray(np.asarray(input, dtype=np.float32))
    Wq = np.ascontiguousarray(np.asarray(Wq, dtype=np.float32))
    Wk = np.ascontiguousarray(np.asarray(Wk, dtype=np.float32))
    Wv = np.ascontiguousarray(np.asarray(Wv, dtype=np.float32))
    B = input.shape[0]
    assert B == 8 and input.shape[1] == N and input.shape[2] == D

    nc = _get_nc()
    in_maps = [
        {"x": input[b], "Wq": Wq, "Wk": Wk, "Wv": Wv} for b in range(B)
    ]
    res = run_bass_kernel_spmd(nc, in_maps, core_ids=list(range(B)))
    return np.stack([res.results[b]["out"] for b in range(B)], axis=0)


# revision 4
# speedup vs baseline: 2.2162x; 2.2162x over previous
"""Bass/Tile kernel for a single causal attention head on 8 trn2 NeuronCores.

Problem: input [8, 2048, 768], Wq/Wk/Wv [768, 64] ->
  O = softmax(causal(Q K^T)/sqrt(64)) V, per batch.  [8, 2048, 64]

Sharding: data-parallel over batch; core b handles batch b. Weights replicated.

v3: all-bf16 matmul pipeline + software-pipelined attention schedule.
  - bf16 matmuls run 1 cyc/row incl. <256-wide outputs (fp32r degrades to 4),
    bf16 PE transposes run 1 cyc/row (fp32 takes 2).
  - default front end (ATTN_X=f32): x stays f32 in DRAM; a gpsimd
    (software-DGE) DMA casts f32->bf16 in flight while loading to SBUF;
    PE transposes the bf16 tiles (psum bf16) and DVE copies to xT at 2x.
  - ATTN_X=bf16: host-casts x, xbar DMA-transpose straight from DRAM
    (no PE transposes at all).
  - attention emitted as stages (pair of k-tiles / diagonal group) with the
    S^T matmuls of stage t+1 issued BEFORE the O^T matmuls of stage t, so
    the exp (ACT) latency of stage t hides behind S^T compute of t+1.
    The diagonal stage (exp + gpsimd affine_select chain) is scheduled
    second in each range so later pair stages cover its longer latency.
    Per-range normalize/output (PE transposes back) is deferred one stage
    into the next range so it never blocks the S/O pipeline.

Per-core dataflow:
  1. xT [768, 2048] bf16 in SBUF (via one of the front ends above).
  2. Projections with concatenated stationary weights [Wq|Wk], [Wv|Wq]:
     [128, 2, 512] psum f32 per range-pair -> 1024-wide cast-copies to
     QK_sb (ACT) / VQ_sb (DVE) bf16.  V^T tiles PE-transposed back to
     V_aug [128, 65] bf16 (ones col makes the O^T matmul emit softmax
     denominators for free).
  3. Attention per 512-wide query range: S^T matmuls (bf16 in, f32 psum),
     one exp per stage on ACT (1/8 scale fused, bf16 out), causal zeroing
     of diagonal blocks via gpsimd affine_select, O^T accumulation in psum.
  4. O^T+rowsum PE-transposed back to [128, 65] f32; out = O * (1/rowsum).

QK_sb/VQ_sb/Vb are double-buffered across reps so rep r+1's projections
don't stall on rep r's attention tail.
"""

import os
import numpy as np

import concourse.tile as tile
from concourse import bacc, mybir
from concourse.bass_utils import run_bass_kernel_spmd
from concourse.masks import make_identity

P = 128
N = 2048
D = 768
H = 64
NT = N // P   # 16 n-tiles
DT = D // P   # 6 d-tiles
W = 512       # q-range width
QR = N // W   # 4 q-ranges
F32 = mybir.dt.float32
BF = mybir.dt.bfloat16

# bf16 x-path: host-cast x to bf16, xbar DMA-transpose straight from DRAM.
BF16_X = os.environ.get("ATTN_X", "f32") == "bf16"

# Three of the four diagonal tiles (widths 512, 384, 128) pack into one
# [128, 1024] psum (bank0: 512 | bank1: 384+128); the 256-wide one (jj=2)
# gets its own narrow tile.  No matmul output crosses a 512-col f32 bank.
DIAG_PACK = {0: (0, 512), 1: (512, 384), 3: (896, 128)}  # jj -> (off, width)
DIAG_TOT = 1024


def build_kernel(reps=1):
    nc = bacc.Bacc(name="attn_head")
    x_d = nc.dram_tensor("x", [N, D], BF if BF16_X else F32,
                         kind="ExternalInput")
    wq_d = nc.dram_tensor("Wq", [D, H], F32, kind="ExternalInput")
    wk_d = nc.dram_tensor("Wk", [D, H], F32, kind="ExternalInput")
    wv_d = nc.dram_tensor("Wv", [D, H], F32, kind="ExternalInput")
    out_d = nc.dram_tensor("out", [N, H], F32, kind="ExternalOutput")

    Exp = mybir.ActivationFunctionType.Exp

    with tile.TileContext(nc) as tc:
        with (
            tc.tile_pool(name="persist", bufs=1) as persist,
            tc.tile_pool(name="xload", bufs=2) as xload,
            tc.tile_pool(name="work", bufs=3) as work,
            tc.tile_pool(name="psum", bufs=1, space="PSUM") as psum,
        ):
            # warm the ACT exp table while DMAs run
            dummy = persist.tile([P, 1], F32)
            nc.vector.memset(dummy[:], 0.0)
            nc.scalar.activation(dummy[:], dummy[:], Exp)

            ident = persist.tile([P, P], F32)
            make_identity(nc, ident[:])
            ident_bf = persist.tile([P, P], BF)
            nc.vector.tensor_copy(out=ident_bf[:], in_=ident[:])

            ones_col = persist.tile([P, 1], F32)
            nc.vector.memset(ones_col[:], 1.0)

            xT = persist.tile([P, DT, N], BF)        # x^T: [d%128, d//128, n]
            QK_sb = persist.tile([P, 2, N], BF)      # rows 0-63 Q^T, 64-127 K^T
            VQ_sb = persist.tile([P, 2, N], BF)      # rows 0-63 V^T, 64-127 Q^T
            Vb = persist.tile([P, 2, NT, H + 1], BF)  # V tiles + ones col
            for s in range(2):
                nc.vector.tensor_copy(
                    out=Vb[:, s, :, H],
                    in_=ones_col[:, 0].to_broadcast((P, NT)),
                )

            # weights as [128, 6, 64]: partition = d%128, tile = d//128.
            # Concatenated pairs [Wq|Wk] and [Wv|Wq] make 128-wide stationary
            # operands: one projection matmul computes two 64-row outputs.
            w_raw = {}
            w_dma = []
            for wname, dram in (("q", wq_d), ("k", wk_d), ("v", wv_d)):
                w_raw[wname] = persist.tile([P, DT, H], F32,
                                            name=f"wraw_{wname}")
                w_dma.append((w_raw[wname], dram))
            w_qk = persist.tile([P, DT, 2 * H], BF)
            w_vq = persist.tile([P, DT, 2 * H], BF)

            def load_weights():
                for w_t, dram in w_dma:
                    nc.sync.dma_start(
                        out=w_t[:],
                        in_=dram[:, :].rearrange("(t p) h -> p t h", p=P),
                    )
                nc.vector.tensor_copy(out=w_qk[:, :, 0:H], in_=w_raw["q"][:])
                nc.vector.tensor_copy(out=w_qk[:, :, H:], in_=w_raw["k"][:])
                nc.vector.tensor_copy(out=w_vq[:, :, 0:H], in_=w_raw["v"][:])
                nc.vector.tensor_copy(out=w_vq[:, :, H:], in_=w_raw["q"][:])

            for rep in range(reps):
                sl = rep % 2
                # ---- x load + transpose to xT ------------------------------
                if BF16_X:
                    # x arrives bf16: xbar DMA-transpose straight from DRAM,
                    # chunked by query range so projections start early
                    for g in range(QR):
                        ns = slice(g * W, (g + 1) * W)
                        for d_i in range(DT):
                            nc.sync.dma_start(
                                out=xT[:, d_i, ns],
                                in_=x_d[ns, d_i * P:(d_i + 1) * P],
                                transpose=True,
                            )
                    if rep == 0:
                        load_weights()
                else:
                    # casting DMA (gpsimd software DGE): f32 DRAM -> bf16
                    # SBUF, one DMA per 4-tile group for pipelining
                    xbs = []
                    for g in range(QR):
                        xb = xload.tile([P, 4, D], BF, tag="xb",
                                        name=f"xb_{rep}_{g}", bufs=2)
                        xbs.append(xb)
                        nc.gpsimd.dma_start(
                            out=xb[:],
                            in_=x_d[g * W:(g + 1) * W, :].rearrange(
                                "(t p) d -> p t d", p=P),
                        )
                    if rep == 0:
                        load_weights()
                    # PE transpose bf16 (1 cyc/row), psum bf16, DVE 2x copy
                    for g in range(QR):
                        for d_i in range(DT):
                            pt = psum.tile([P, 4, P], BF, tag="mm", bufs=2)
                            for i in range(4):
                                nc.tensor.transpose(
                                    pt[:, i, :],
                                    xbs[g][:, i, d_i * P:(d_i + 1) * P],
                                    ident_bf[:],
                                )
                            nc.vector.tensor_copy(
                                out=xT[:, d_i, g * W:(g + 1) * W],
                                in_=pt[:])

                # ---- projections (packed weights, range-pair tiles) --------
                for h2 in range(2):  # ranges (2*h2, 2*h2+1)
                    ds = slice(2 * h2 * W, (2 * h2 + 2) * W)
                    pqk2 = psum.tile([P, 2, W], F32, tag="mm", bufs=2,
                                     name=f"pqk2_{rep}_{h2}")
                    pvq2 = psum.tile([P, 2, W], F32, tag="mm", bufs=2,
                                     name=f"pvq2_{rep}_{h2}")
                    for d_i in range(DT):
                        kw = dict(start=(d_i == 0), stop=(d_i == DT - 1),
                                  skip_group_check=True)
                        for u in range(2):
                            rhs = xT[:, d_i, (2 * h2 + u) * W:
                                     (2 * h2 + u + 1) * W]
                            nc.tensor.matmul(pqk2[:, u, :], w_qk[:, d_i],
                                             rhs, **kw)
                    nc.scalar.copy(out=QK_sb[:, sl, ds], in_=pqk2[:])
                    for d_i in range(DT):
                        kw = dict(start=(d_i == 0), stop=(d_i == DT - 1),
                                  skip_group_check=True)
                        for u in range(2):
                            rhs = xT[:, d_i, (2 * h2 + u) * W:
                                     (2 * h2 + u + 1) * W]
                            nc.tensor.matmul(pvq2[:, u, :], w_vq[:, d_i],
                                             rhs, **kw)
                    nc.vector.tensor_copy(out=VQ_sb[:, sl, ds], in_=pvq2[:])
                    # V^T tiles -> V_aug [128, 65] per k-tile (batched copy)
                    pvt = psum.tile([P, 8, H], BF, tag="proj", bufs=2,
                                    name=f"pvt_{rep}_{h2}")
                    for i in range(8):
                        nc.tensor.transpose(
                            pvt[:, i, :],
                            VQ_sb[0:H, sl,
                                  2 * h2 * W + i * P:2 * h2 * W + (i + 1) * P],
                            ident_bf[:H, :H],
                        )
                    nc.vector.tensor_copy(
                        out=Vb[:, sl, 8 * h2:8 * h2 + 8, 0:H], in_=pvt[:])

                # ---- attention: software-pipelined stages ------------------
                KTd = QK_sb[H:P, sl, :]   # K^T on partitions 64-127
                QTd = VQ_sb[H:P, sl, :]   # Q^T duplicate on partitions 64-127
                po_tiles = {}

                def po_for(r):
                    if r not in po_tiles:
                        po_tiles[r] = psum.tile([H + 1, W], F32, tag="po",
                                                bufs=2, name=f"po_{rep}_{r}")
                    return po_tiles[r]

                def make_pair(r, jp):
                    ps2 = {}

                    def emit_S():
                        ps2["t"] = psum.tile([P, 2, W], F32, tag="mm",
                                             bufs=2,
                                             name=f"ps2_{rep}_{r}_{jp}")
                        ps2["e"] = work.tile([P, 2, W], BF, tag="es2",
                                             bufs=6,
                                             name=f"es2_{rep}_{r}_{jp}")
                        qs = slice(r * W, (r + 1) * W)
                        for u in range(2):
                            j = 2 * jp + u
                            nc.tensor.matmul(
                                ps2["t"][:, u, :],
                                KTd[:, j * P:(j + 1) * P],
                                QTd[:, qs], start=True, stop=True,
                            )
                        nc.scalar.activation(ps2["e"][:], ps2["t"][:], Exp,
                                             scale=0.125)

                    def emit_O(first, last):
                        po = po_for(r)
                        for u in range(2):
                            j = 2 * jp + u
                            nc.tensor.matmul(
                                po[:], Vb[:, sl, j, :], ps2["e"][:, u, :],
                                start=(first and u == 0),
                                stop=(last and u == 1),
                                skip_group_check=True,
                            )

                    return emit_S, emit_O

                def make_diag(r):
                    st = {}

                    def emit_S():
                        st["psd"] = psum.tile([P, DIAG_TOT], F32, tag="mm",
                                              bufs=2, name=f"psd_{rep}_{r}")
                        st["esd"] = work.tile([P, DIAG_TOT], BF, tag="esd",
                                              bufs=4, name=f"esd_{rep}_{r}")
                        st["ps1"] = psum.tile([P, 256], F32, tag="proj",
                                              bufs=2, name=f"ps1_{rep}_{r}")
                        st["es1"] = work.tile([P, 256], BF, tag="es1",
                                              bufs=4, name=f"es1_{rep}_{r}")
                        psd, esd = st["psd"], st["esd"]
                        ps1, es1 = st["ps1"], st["es1"]
                        for jj in range(4):
                            j = 4 * r + jj
                            if jj == 2:
                                sv = ps1[:, :]
                            else:
                                poff, wd = DIAG_PACK[jj]
                                sv = psd[:, poff:poff + wd]
                            nc.tensor.matmul(
                                sv,
                                KTd[:, j * P:(j + 1) * P],
                                QTd[:, r * W + jj * P:(r + 1) * W],
                                start=True, stop=True,
                            )
                        nc.scalar.activation(esd[:, 0:W], psd[:, 0:W], Exp,
                                             scale=0.125)
                        nc.scalar.activation(esd[:, W:], psd[:, W:], Exp,
                                             scale=0.125)
                        nc.scalar.activation(es1[:], ps1[:], Exp, scale=0.125)
                        # zero the invalid (q < k) half of each diagonal
                        # 128x128 block post-exp on the idle GPSIMD engine
                        for jj in range(4):
                            ev = es1[:, 0:P] if jj == 2 else (
                                esd[:, DIAG_PACK[jj][0]:
                                    DIAG_PACK[jj][0] + P])
                            nc.gpsimd.affine_select(
                                out=ev, in_=ev,
                                compare_op=mybir.AluOpType.is_ge,
                                fill=0.0, base=0,
                                pattern=[[1, P]], channel_multiplier=-1,
                            )

                    def emit_O(first, last):
                        po = po_for(r)
                        esd, es1 = st["esd"], st["es1"]
                        for jj in range(4):
                            j = 4 * r + jj
                            if jj == 2:
                                rhs = es1[:, :]
                            else:
                                poff, wd = DIAG_PACK[jj]
                                rhs = esd[:, poff:poff + wd]
                            nc.tensor.matmul(
                                po[:, jj * P:],
                                Vb[:, sl, j, :],
                                rhs,
                                start=(first and jj == 0),
                                stop=(last and jj == 3),
                                skip_group_check=True,
                            )

                    return emit_S, emit_O

                def final_dve(r):
                    ot = work.tile([H + 1, W], F32, tag="ot", bufs=4,
                                   name=f"ot_{rep}_{r}")
                    nc.vector.tensor_copy(out=ot[:], in_=po_tiles[r][:])
                    return ot

                def final_pe(r, ot):
                    pf = psum.tile([P, 4, H + 1], F32, tag="proj", bufs=2,
                                   name=f"pf_{rep}_{r}")
                    for i in range(4):
                        nt = r * 4 + i
                        nc.tensor.transpose(
                            pf[:, i, :], ot[:, i * P:(i + 1) * P],
                            ident[:H + 1, :H + 1],
                        )
                        rs = work.tile([P, 1], F32, tag="rs",
                                       name=f"rs_{rep}_{nt}")
                        nc.vector.reciprocal(rs[:], pf[:, i, H:H + 1])
                        ob = work.tile([P, H], F32, tag="ob",
                                       name=f"ob_{rep}_{nt}", bufs=4)
                        nc.vector.tensor_scalar_mul(
                            ob[:], pf[:, i, 0:H], rs[:]
                        )
                        nc.sync.dma_start(
                            out=out_d[nt * P:(nt + 1) * P, :],
                            in_=ob[:],
                        )

                # build stage list: diag scheduled second in each range so
                # pair stages cover its exp+affine latency (r=0: alone)
                stages = []  # (emit_S, emit_O, r, first, last)
                for r in range(QR):
                    pairs = [make_pair(r, jp) for jp in range(2 * r)]
                    dstage = make_diag(r)
                    if pairs:
                        order = [pairs[0], dstage] + pairs[1:]
                    else:
                        order = [dstage]
                    for i, (eS, eO) in enumerate(order):
                        stages.append((eS, eO, r, i == 0,
                                       i == len(order) - 1))

                stages[0][0]()          # S of stage 0
                pending = None          # (r, ot) awaiting final_pe
                for t, (eS, eO, r, first, last) in enumerate(stages):
                    if t + 1 < len(stages):
                        stages[t + 1][0]()   # S of next stage
                    eO(first, last)
                    if pending is not None:
                        final_pe(*pending)
                        pending = None
                    if last:
                        pending = (r, final_dve(r))
                if pending is not None:
                    final_pe(*pending)

    nc.compile()
    return nc


_NC_CACHE = {}


def _get_nc(reps=1):
    if reps not in _NC_CACHE:
        _NC_CACHE[reps] = build_kernel(reps)
    return _NC_CACHE[reps]


def kernel(input, Wq, Wk, Wv, **_unused):
    if BF16_X:
        import ml_dtypes
        input = np.ascontiguousarray(
            np.asarray(input).astype(ml_dtypes.bfloat16))
    else:
        input = np.ascontiguousarray(np.asarray(input, dtype=np.float32))
    Wq = np.ascontiguousarray(np.asarray(Wq, dtype=np.float32))
    Wk = np.ascontiguousarray(np.asarray(Wk, dtype=np.float32))
    Wv = np.ascontiguousarray(np.asarray(Wv, dtype=np.float32))
    B = input.shape[0]
    assert B == 8 and input.shape[1] == N and input.shape[2] == D

    nc = _get_nc()
    in_maps = [
        {"x": input[b], "Wq": Wq, "Wk": Wk, "Wv": Wv} for b in range(B)
    ]
    res = run_bass_kernel_spmd(nc, in_maps, core_ids=list(range(B)))
    return np.stack([res.results[b]["out"] for b in range(B)], axis=0)
